# revision 2
# baseline (speedup 1.0000x reference)
"""IoU loss kernel for Trainium2, data-parallel over the batch dim on 8 cores.

Math (per reference):
    probs = softmax(inputs, axis=1)                       # (8, 13, 800, 800)
    intersection = sum_pix probs[b, t, h, w]
    total = probs.sum() + Npix                            # probs.sum() == Npix
    out = 1 - (intersection + smooth) / (total - intersection + smooth)

v6 device kernel (per core, one batch item), raw Bass. Per chunk j
(deeply multi-buffered; all five engines in a DMA-bound pipeline):
    DMA    X = x chunk (host pre-packed chunk-contiguous f32)
    ACT    E = exp(X)  (bf16)
    PE     D = sum_c E_c  -- 13 identity-weight matmuls accumulating in
           one PSUM bank (f32); runs in parallel with GPSIMD/DVE below
    GPSIMD W[i] = E[2i]  (one strided copy: mux workspace prefill)
    DVE    4-level bit-mux of W/E -> W0 = E[target]  (batched
           copy_predicated, masks precomputed full-width upfront)
    ACT    R = exp(-ln D) = 1/D  (both funcs share one ACT table set)
    DVE    acc[:, j] += W0 * R  (stt, fused mult + free-dim accumulate;
           woven one chunk behind the mux so R's latency is hidden)
Host sums the 8 x 128 x NCHUNK partials and forms the scalar.
"""

import numpy as np

_BS, _C, _H, _W = 8, 13, 800, 800
_P = 128
_FREE = (_H * _W) // _P  # 5000
_CHUNKS = [200] * 25
assert sum(_CHUNKS) == _FREE
_NCHUNK = len(_CHUNKS)
_NMAX = max(_CHUNKS)
_OFFS = [sum(_CHUNKS[:j]) for j in range(_NCHUNK)]
_NBUF = 7       # X (f32 input) buffers
_EBUF = 9       # E (exp output) buffers
_DBANK = 8      # PSUM banks cycling for D
_WBUF = 4       # W (mux workspace) / R buffers
_NCORES = 8
_NPIX = _BS * _H * _W    # 5120000

_cached = {}


def _build_program():
    from contextlib import ExitStack

    import concourse.bass as bass
    import concourse.mybir as mybir

    f32 = mybir.dt.float32
    bf16 = mybir.dt.bfloat16
    u8 = mybir.dt.uint8
    Alu = mybir.AluOpType
    Act = mybir.ActivationFunctionType

    nc = bass.Bass(trn_type="TRN2")
    x = nc.declare_dram_parameter("x", [_NCHUNK, _P, _C, _NMAX], f32,
                                  isOutput=False)
    t = nc.declare_dram_parameter("t", [_P, _FREE], u8, isOutput=False)
    ident = nc.declare_dram_parameter("ident", [_P, _P], bf16, isOutput=False)
    part = nc.declare_dram_parameter("part", [_P, _NCHUNK], f32, isOutput=True)

    ctx = ExitStack()
    with ctx:
        T = ctx.enter_context(nc.sbuf_tensor("T", [_P, _FREE], u8))
        Id = ctx.enter_context(nc.sbuf_tensor("Id", [_P, _P], bf16))
        M = [ctx.enter_context(nc.sbuf_tensor(f"M{k}", [_P, _FREE], u8))
             for k in range(4)]
        acc = ctx.enter_context(nc.sbuf_tensor("acc", [_P, _NCHUNK], f32))
        dummy = ctx.enter_context(nc.sbuf_tensor("ttr_dummy", [_P, 1], f32))
        X = [ctx.enter_context(nc.sbuf_tensor(f"X{i}", [_P, _C, _NMAX], f32))
             for i in range(_NBUF)]
        E = [ctx.enter_context(nc.sbuf_tensor(f"E{i}", [_P, _C, _NMAX], bf16))
             for i in range(_EBUF)]
        D = [ctx.enter_context(nc.psum_tensor(f"D{i}", [_P, _NMAX], f32))
             for i in range(_DBANK)]
        W = [ctx.enter_context(nc.sbuf_tensor(f"W{i}", [_P, 7, _NMAX], bf16))
             for i in range(_WBUF)]
        L = ctx.enter_context(nc.sbuf_tensor("Lbuf", [_P, _NMAX], f32))
        R = [ctx.enter_context(nc.sbuf_tensor(f"R{i}", [_P, _NMAX], f32))
             for i in range(_WBUF)]

        block = ctx.enter_context(nc.Block())
        dma_t = ctx.enter_context(nc.semaphore("dma_t"))
        dma_id = ctx.enter_context(nc.semaphore("dma_id"))
        dma_x = [ctx.enter_context(nc.semaphore(f"dma_x{i}"))
                 for i in range(_NBUF)]
        dma_out = ctx.enter_context(nc.semaphore("dma_out"))
        s_exp = ctx.enter_context(nc.semaphore("s_exp"))
        s_pe = ctx.enter_context(nc.semaphore("s_pe"))
        s_gpw = ctx.enter_context(nc.semaphore("s_gpw"))
        s_r = ctx.enter_context(nc.semaphore("s_r"))
        s_ttr = ctx.enter_context(nc.semaphore("s_ttr"))

        @block.sync
        def _(sync):
            sync.dma_start(out=Id[:, :], in_=ident[:, :]).then_inc(dma_id, 16)
            sync.dma_start(out=T[:, :], in_=t[:, :]).then_inc(dma_t, 16)
            for j in range(_NCHUNK):
                b = j % _NBUF
                rnd = j // _NBUF
                n = _CHUNKS[j]
                if j >= _NBUF:
                    # X[b] reread by exp of chunk j-NBUF; exp-done implies
                    # X free (and the slot's previous DMA completed).
                    sync.wait_ge(s_exp, j - _NBUF + 1)
                    sync.wait_ge(dma_x[b], 16 * rnd)
                sync.dma_start(
                    out=X[b][:, :, :n], in_=x[j, :, :, :n],
                ).then_inc(dma_x[b], 16)
            sync.wait_ge(s_ttr, _NCHUNK)
            sync.dma_start(out=part[:, :], in_=acc[:, :]).then_inc(dma_out, 16)
            sync.wait_ge(dma_out, 16)

        @block.scalar
        def _(scalar):
            def _r_ops(j):
                # R_j = 1/D_j as exp(-ln(D_j)); Ln and Exp share one ACT
                # table set, so no per-chunk set reloads
                n = _CHUNKS[j]
                scalar.wait_ge(s_pe, j + 1)
                scalar.activation(out=L[:, :n], in_=D[j % _DBANK][:, :n],
                                  func=Act.Ln)
                scalar.activation(out=R[j % _WBUF][:, :n], in_=L[:, :n],
                                  func=Act.Exp, scale=-1.0).then_inc(s_r, 1)

            for j in range(_NCHUNK):
                b = j % _NBUF
                be = j % _EBUF
                n = _CHUNKS[j]
                scalar.wait_ge(dma_x[b], 16 * (j // _NBUF + 1))
                if j >= _EBUF:
                    # E[be] readers from chunk j-EBUF: PE matmuls, GPSIMD
                    # W-prefill, DVE mux — all implied done by s_ttr.
                    scalar.wait_ge(s_ttr, j - _EBUF + 1)
                scalar.activation(
                    out=E[be][:, :, :n], in_=X[b][:, :, :n], func=Act.Exp
                ).then_inc(s_exp, 1)
                if j >= 1:
                    _r_ops(j - 1)
            _r_ops(_NCHUNK - 1)

        @block.gpsimd
        def _(gpsimd):
            for j in range(_NCHUNK):
                be = j % _EBUF
                w = j % _WBUF
                n = _CHUNKS[j]
                gpsimd.wait_ge(s_exp, j + 1)
                if j >= _WBUF:
                    # W[w] read by DVE until the stt of chunk j-WBUF
                    gpsimd.wait_ge(s_ttr, j - _WBUF + 1)
                # W[i] = E[2i] prefill, one strided copy (evens + class 12)
                gpsimd.tensor_copy(
                    W[w][:, :, :n], E[be][:, 0:13:2, :n],
                ).then_inc(s_gpw, 1)

        @block.tensor
        def _(tensor):
            tensor.wait_ge(dma_id, 16)
            for j in range(_NCHUNK):
                be = j % _EBUF
                d = j % _DBANK
                n = _CHUNKS[j]
                tensor.wait_ge(s_exp, j + 1)
                if j >= _DBANK:
                    # D[d] is read by ACT's ln of chunk j-DBANK (s_r covers)
                    tensor.wait_ge(s_r, j - _DBANK + 1)
                # D = sum_c E_c: 13 identity matmuls accumulating in PSUM
                for c in range(_C):
                    mm = tensor.matmul(
                        out=D[d][:, :n], lhsT=Id[:, :],
                        rhs=E[be][:, c, :n],
                        start=(c == 0), stop=(c == _C - 1),
                    )
                mm.then_inc(s_pe, 1)

        @block.vector
        def _(vector):
            vector.wait_ge(dma_t, 16)
            # full-width bit masks, computed once upfront
            for k in range(4):
                vector.tensor_scalar(M[k][:, :], T[:, :], 1 << k, None,
                                     Alu.bitwise_and)

            def _stt(j):
                n = _CHUNKS[j]
                w = j % _WBUF
                vector.wait_ge(s_r, j + 1)
                vector.scalar_tensor_tensor(
                    out=dummy[:, :].broadcast_to((_P, n)),
                    in0=W[w][:, 0, :n],
                    scalar=1.0,
                    in1=R[w][:, :n],
                    op0=Alu.bypass,
                    op1=Alu.mult,
                    accum_out=acc[:, j:j + 1],
                ).then_inc(s_ttr, 1)

            for j in range(_NCHUNK):
                w = j % _WBUF
                n = _CHUNKS[j]
                Eb = E[j % _EBUF]
                Wb = W[w]
                sl = slice(_OFFS[j], _OFFS[j] + n)
                m = [M[k][:, None, sl] for k in range(4)]
                # mux on the GPSIMD-prefilled W (E stays intact for the PE)
                vector.wait_ge(s_gpw, j + 1)
                # mux L0 (bit0): W[i] <- E[2i+1], i<6; W6=E12 rides
                vector.copy_predicated(
                    Wb[:, 0:6, :n], m[0].broadcast_to((_P, 6, n)),
                    Eb[:, 1:12:2, :n])
                # mux L1 (bit1): (W0,W1) (W2,W3) (W4,W5)
                vector.copy_predicated(
                    Wb[:, 0:6:2, :n], m[1].broadcast_to((_P, 3, n)),
                    Wb[:, 1:6:2, :n])
                # mux L2 (bit2): (W0,W2) (W4,W6)
                vector.copy_predicated(
                    Wb[:, 0:7:4, :n], m[2].broadcast_to((_P, 2, n)),
                    Wb[:, 2:7:4, :n])
                # mux L3 (bit3): (W0,W4)
                vector.copy_predicated(Wb[:, 0, :n], M[3][:, sl],
                                       Wb[:, 4, :n])
                # accumulate the previous chunk (R arrives one chunk late)
                if j >= 1:
                    _stt(j - 1)
            _stt(_NCHUNK - 1)

    return nc


def _get_program():
    if "nc" not in _cached:
        _cached["nc"] = _build_program()
    return _cached["nc"]


def _identity_bf16():
    import ml_dtypes

    return np.eye(_P, dtype=np.float32).astype(ml_dtypes.bfloat16)


def _make_in_maps(inputs, targets):
    in_maps = []
    ident = _identity_bf16()
    for b in range(_NCORES):
        xb = np.ascontiguousarray(inputs[b]).reshape(_C, _P, _NCHUNK,
                                                      _NMAX)
        m = {"x": np.ascontiguousarray(xb.transpose(2, 1, 0, 3))}
        m["t"] = np.ascontiguousarray(targets[b]).astype(np.uint8).reshape(
            _P, _FREE)
        m["ident"] = ident
        in_maps.append(m)
    return in_maps


def _finalize(parts, smooth):
    inter = 0.0
    for p in parts:
        inter += float(np.sum(np.asarray(p).astype(np.float64)))
    s = float(smooth)
    total = 2.0 * float(_NPIX)
    union = total - inter
    out = 1.0 - (inter + s) / (union + s)
    return np.asarray(np.float32(out))


def kernel(inputs, targets, smooth):
    from concourse.bass_utils import run_bass_kernel_spmd

    nc = _get_program()
    in_maps = _make_in_maps(np.asarray(inputs), np.asarray(targets))
    res = run_bass_kernel_spmd(nc, in_maps, list(range(_NCORES)))
    return _finalize([res.results[b]["part"] for b in range(_NCORES)], smooth)


# revision 3
# speedup vs baseline: 1.1376x; 1.1376x over previous
"""IoU loss kernel for Trainium2, data-parallel over the batch dim on 8 cores.

Math (per reference):
    probs = softmax(inputs, axis=1)                       # (8, 13, 800, 800)
    intersection = sum_pix probs[b, t, h, w]
    total = probs.sum() + Npix                            # probs.sum() == Npix
    out = 1 - (intersection + smooth) / (total - intersection + smooth)

v6 device kernel (per core, one batch item), raw Bass. Per chunk j
(deeply multi-buffered; all five engines in a DMA-bound pipeline):
    DMA    X = x chunk (host pre-packed chunk-contiguous f32)
    ACT    E = exp(X)  (bf16)
    PE     D = sum_c E_c  -- 13 identity-weight matmuls accumulating in
           one PSUM bank (f32); runs in parallel with GPSIMD/DVE below
    GPSIMD W[i] = E[2i]  (one strided copy: mux workspace prefill)
    DVE    4-level bit-mux of W/E -> W0 = E[target]  (batched
           copy_predicated, masks precomputed full-width upfront)
    ACT    R = exp(-ln D) = 1/D  (both funcs share one ACT table set)
    DVE    acc[:, j] += W0 * R  (stt, fused mult + free-dim accumulate;
           woven one chunk behind the mux so R's latency is hidden)
Host sums the 8 x 128 x NCHUNK partials and forms the scalar.
"""

import numpy as np

_BS, _C, _H, _W = 8, 13, 800, 800
_P = 128
_FREE = (_H * _W) // _P  # 5000
_CHUNKS = [100] * 50
assert sum(_CHUNKS) == _FREE
_NCHUNK = len(_CHUNKS)
_NMAX = max(_CHUNKS)
_OFFS = [sum(_CHUNKS[:j]) for j in range(_NCHUNK)]
_NBUF = 10      # X (f32 input) buffers
_EBUF = 12      # E (exp output) buffers
_DBANK = 8      # PSUM banks cycling for D
_WBUF = 4       # W (mux workspace) / R buffers
_NCORES = 8
_NPIX = _BS * _H * _W    # 5120000

_cached = {}


def _build_program():
    from contextlib import ExitStack

    import concourse.bass as bass
    import concourse.mybir as mybir

    f32 = mybir.dt.float32
    bf16 = mybir.dt.bfloat16
    u8 = mybir.dt.uint8
    Alu = mybir.AluOpType
    Act = mybir.ActivationFunctionType

    nc = bass.Bass(trn_type="TRN2")
    x = nc.declare_dram_parameter("x", [_NCHUNK, _P, _C, _NMAX], f32,
                                  isOutput=False)
    t = nc.declare_dram_parameter("t", [_P, _FREE], u8, isOutput=False)
    ident = nc.declare_dram_parameter("ident", [_P, _P], bf16, isOutput=False)
    part = nc.declare_dram_parameter("part", [_P, _NCHUNK], f32, isOutput=True)

    ctx = ExitStack()
    with ctx:
        T = ctx.enter_context(nc.sbuf_tensor("T", [_P, _FREE], u8))
        Id = ctx.enter_context(nc.sbuf_tensor("Id", [_P, _P], bf16))
        M = [ctx.enter_context(nc.sbuf_tensor(f"M{k}", [_P, _FREE], u8))
             for k in range(4)]
        acc = ctx.enter_context(nc.sbuf_tensor("acc", [_P, _NCHUNK], f32))
        dummy = ctx.enter_context(nc.sbuf_tensor("ttr_dummy", [_P, 1], f32))
        X = [ctx.enter_context(nc.sbuf_tensor(f"X{i}", [_P, _C, _NMAX], f32))
             for i in range(_NBUF)]
        E = [ctx.enter_context(nc.sbuf_tensor(f"E{i}", [_P, _C, _NMAX], bf16))
             for i in range(_EBUF)]
        D = [ctx.enter_context(nc.psum_tensor(f"D{i}", [_P, _NMAX], f32))
             for i in range(_DBANK)]
        W = [ctx.enter_context(nc.sbuf_tensor(f"W{i}", [_P, 7, _NMAX], bf16))
             for i in range(_WBUF)]
        L = ctx.enter_context(nc.sbuf_tensor("Lbuf", [_P, _NMAX], f32))
        R = [ctx.enter_context(nc.sbuf_tensor(f"R{i}", [_P, _NMAX], f32))
             for i in range(_WBUF)]

        block = ctx.enter_context(nc.Block())
        dma_t = ctx.enter_context(nc.semaphore("dma_t"))
        dma_id = ctx.enter_context(nc.semaphore("dma_id"))
        dma_x = [ctx.enter_context(nc.semaphore(f"dma_x{i}"))
                 for i in range(_NBUF)]
        dma_out = ctx.enter_context(nc.semaphore("dma_out"))
        s_exp = ctx.enter_context(nc.semaphore("s_exp"))
        s_pe = ctx.enter_context(nc.semaphore("s_pe"))
        s_gpw = ctx.enter_context(nc.semaphore("s_gpw"))
        s_r = ctx.enter_context(nc.semaphore("s_r"))
        s_ttr = ctx.enter_context(nc.semaphore("s_ttr"))

        @block.sync
        def _(sync):
            sync.dma_start(out=Id[:, :], in_=ident[:, :]).then_inc(dma_id, 16)
            sync.dma_start(out=T[:, :], in_=t[:, :]).then_inc(dma_t, 16)
            for j in range(_NCHUNK):
                b = j % _NBUF
                rnd = j // _NBUF
                n = _CHUNKS[j]
                if j >= _NBUF:
                    # X[b] reread by exp of chunk j-NBUF; exp-done implies
                    # X free (and the slot's previous DMA completed).
                    sync.wait_ge(s_exp, j - _NBUF + 1)
                    sync.wait_ge(dma_x[b], 16 * rnd)
                sync.dma_start(
                    out=X[b][:, :, :n], in_=x[j, :, :, :n],
                ).then_inc(dma_x[b], 16)
            sync.wait_ge(s_ttr, _NCHUNK)
            sync.dma_start(out=part[:, :], in_=acc[:, :]).then_inc(dma_out, 16)
            sync.wait_ge(dma_out, 16)

        @block.scalar
        def _(scalar):
            def _r_ops(j):
                # R_j = 1/D_j as exp(-ln(D_j)); Ln and Exp share one ACT
                # table set, so no per-chunk set reloads
                n = _CHUNKS[j]
                scalar.wait_ge(s_pe, j + 1)
                scalar.activation(out=L[:, :n], in_=D[j % _DBANK][:, :n],
                                  func=Act.Ln)
                scalar.activation(out=R[j % _WBUF][:, :n], in_=L[:, :n],
                                  func=Act.Exp, scale=-1.0).then_inc(s_r, 1)

            for j in range(_NCHUNK):
                b = j % _NBUF
                be = j % _EBUF
                n = _CHUNKS[j]
                scalar.wait_ge(dma_x[b], 16 * (j // _NBUF + 1))
                if j >= _EBUF:
                    # E[be] readers from chunk j-EBUF: PE matmuls, GPSIMD
                    # W-prefill, DVE mux — all implied done by s_ttr.
                    scalar.wait_ge(s_ttr, j - _EBUF + 1)
                scalar.activation(
                    out=E[be][:, :, :n], in_=X[b][:, :, :n], func=Act.Exp
                ).then_inc(s_exp, 1)
                if j >= 1:
                    _r_ops(j - 1)
            _r_ops(_NCHUNK - 1)

        @block.gpsimd
        def _(gpsimd):
            for j in range(_NCHUNK):
                be = j % _EBUF
                w = j % _WBUF
                n = _CHUNKS[j]
                gpsimd.wait_ge(s_exp, j + 1)
                if j >= _WBUF:
                    # W[w] read by DVE until the stt of chunk j-WBUF
                    gpsimd.wait_ge(s_ttr, j - _WBUF + 1)
                # W[i] = E[2i] prefill, one strided copy (evens + class 12)
                gpsimd.tensor_copy(
                    W[w][:, :, :n], E[be][:, 0:13:2, :n],
                ).then_inc(s_gpw, 1)

        @block.tensor
        def _(tensor):
            tensor.wait_ge(dma_id, 16)
            for j in range(_NCHUNK):
                be = j % _EBUF
                d = j % _DBANK
                n = _CHUNKS[j]
                tensor.wait_ge(s_exp, j + 1)
                if j >= _DBANK:
                    # D[d] is read by ACT's ln of chunk j-DBANK (s_r covers)
                    tensor.wait_ge(s_r, j - _DBANK + 1)
                # D = sum_c E_c: 13 identity matmuls accumulating in PSUM
                for c in range(_C):
                    mm = tensor.matmul(
                        out=D[d][:, :n], lhsT=Id[:, :],
                        rhs=E[be][:, c, :n],
                        start=(c == 0), stop=(c == _C - 1),
                    )
                mm.then_inc(s_pe, 1)

        @block.vector
        def _(vector):
            vector.wait_ge(dma_t, 16)
            # full-width bit masks, computed once upfront
            for k in range(4):
                vector.tensor_scalar(M[k][:, :], T[:, :], 1 << k, None,
                                     Alu.bitwise_and)

            def _stt(j):
                n = _CHUNKS[j]
                w = j % _WBUF
                vector.wait_ge(s_r, j + 1)
                vector.scalar_tensor_tensor(
                    out=dummy[:, :].broadcast_to((_P, n)),
                    in0=W[w][:, 0, :n],
                    scalar=1.0,
                    in1=R[w][:, :n],
                    op0=Alu.bypass,
                    op1=Alu.mult,
                    accum_out=acc[:, j:j + 1],
                ).then_inc(s_ttr, 1)

            for j in range(_NCHUNK):
                w = j % _WBUF
                n = _CHUNKS[j]
                Eb = E[j % _EBUF]
                Wb = W[w]
                sl = slice(_OFFS[j], _OFFS[j] + n)
                m = [M[k][:, None, sl] for k in range(4)]
                # mux on the GPSIMD-prefilled W (E stays intact for the PE)
                vector.wait_ge(s_gpw, j + 1)
                # mux L0 (bit0): W[i] <- E[2i+1], i<6; W6=E12 rides
                vector.copy_predicated(
                    Wb[:, 0:6, :n], m[0].broadcast_to((_P, 6, n)),
                    Eb[:, 1:12:2, :n])
                # mux L1 (bit1): (W0,W1) (W2,W3) (W4,W5)
                vector.copy_predicated(
                    Wb[:, 0:6:2, :n], m[1].broadcast_to((_P, 3, n)),
                    Wb[:, 1:6:2, :n])
                # mux L2 (bit2): (W0,W2) (W4,W6)
                vector.copy_predicated(
                    Wb[:, 0:7:4, :n], m[2].broadcast_to((_P, 2, n)),
                    Wb[:, 2:7:4, :n])
                # mux L3 (bit3): (W0,W4)
                vector.copy_predicated(Wb[:, 0, :n], M[3][:, sl],
                                       Wb[:, 4, :n])
                # accumulate the previous chunk (R arrives one chunk late)
                if j >= 1:
                    _stt(j - 1)
            _stt(_NCHUNK - 1)

    return nc


def _get_program():
    if "nc" not in _cached:
        _cached["nc"] = _build_program()
    return _cached["nc"]


def _identity_bf16():
    import ml_dtypes

    return np.eye(_P, dtype=np.float32).astype(ml_dtypes.bfloat16)


def _make_in_maps(inputs, targets):
    in_maps = []
    ident = _identity_bf16()
    for b in range(_NCORES):
        xb = np.ascontiguousarray(inputs[b]).reshape(_C, _P, _NCHUNK,
                                                      _NMAX)
        m = {"x": np.ascontiguousarray(xb.transpose(2, 1, 0, 3))}
        m["t"] = np.ascontiguousarray(targets[b]).astype(np.uint8).reshape(
            _P, _FREE)
        m["ident"] = ident
        in_maps.append(m)
    return in_maps


def _finalize(parts, smooth):
    inter = 0.0
    for p in parts:
        inter += float(np.sum(np.asarray(p).astype(np.float64)))
    s = float(smooth)
    total = 2.0 * float(_NPIX)
    union = total - inter
    out = 1.0 - (inter + s) / (union + s)
    return np.asarray(np.float32(out))


def kernel(inputs, targets, smooth):
    from concourse.bass_utils import run_bass_kernel_spmd

    nc = _get_program()
    in_maps = _make_in_maps(np.asarray(inputs), np.asarray(targets))
    res = run_bass_kernel_spmd(nc, in_maps, list(range(_NCORES)))
    return _finalize([res.results[b]["part"] for b in range(_NCORES)], smooth)
